# revision 3
# baseline (speedup 1.0000x reference)
# Trainium2 Bass kernel for batched CG combine:
#   out[i, p, a, b] = sum_{m,n} A[i, m, a] * B[i, n, b] * C[m, n, p]
# A: (600000, 3, 3) f32, B: (600000, 5, 5) f32, C: (3, 5, 5) f32
# out: (600000, 5, 15) f32
#
# Algorithm: exact rank-8 CP decomposition C[m,n,p] = sum_r U[m,r] V[n,r] W[p,r].
# Per atom-tile (atoms on the free dim, NT=512 = one PSUM bank):
#   AU_E[(r,a,b), i] = sum_m (U[m,r] * dirac_a) A[(m,a), i]      (PE matmul, K=9)
#   BV_E[(r,a,b), i] = sum_n (V[n,r] * dirac_b) B[(n,b), i]      (PE matmul, K=25)
#   P = AU_E * BV_E                                              (DVE multiply)
#   out[(p,a,b), i] = sum_(r,a,b) (W[p,r] dirac_a dirac_b) P     (PE matmul, K=120)
#
# Key structure (vs. the 283us baseline):
#  - fp16 inputs and outputs over DMA (in 5.1MB + out 11.3MB per core vs
#    39.6MB), tightly packed [9, NPC]/[25, NPC]/[75, NPC] layouts.
#  - NT=512 fills a PSUM bank exactly; atoms padded to 147 tiles/core.
#  - BV lands in a 2-bank PSUM tile [120, 1024]; ONE DVE tensor_mul per
#    PAIR of tiles (FD=1024) halves the per-op fixed cost of the multiply.
#  - Copy assignment balanced per the engine cost model: DVE (0.96GHz,
#    captive to the mul) takes one au-copy per pair; ACT (1.2GHz) takes
#    the other au-copy and both out-copies.
#  - PSUM budget: au 2 banks + bv-wide 4 banks + o 2 banks = 8 exactly.
#
# Sharding: data-parallel over atoms, 75000 per core across 8 cores.

import numpy as np

N_ATOMS = 600000
NCORES = 8
NPC = N_ATOMS // NCORES   # 75000 real atoms per core
NT = 512                  # atoms per tile = one PSUM bank of f32
T = 147                   # tiles per core
NPAD = NT * T             # 75264 padded atoms per core
CH = 21                   # tiles per DMA chunk (147 = 7 * 21)
NCHUNK = T // CH          # 7 chunks per core

R = 8  # CP rank (exact for this CG tensor)

U = np.array([[0.2419016152442985, 0.6625062831986197, -0.8309374270990885, 0.3998142823675103, -0.5651140448972596, -0.34640840162110975, 0.7646485241540064, -0.0981640650113134], [0.9679329076741274, -0.6672684032643771, -0.5353370910241713, -0.9127024843358726, 0.26799289625560263, 0.8715541794335616, -0.5278177753574712, -0.018552310924435454], [0.06774581008230969, 0.3403502647675755, 0.1515163067782647, -0.08439617705843598, 0.7802729803193187, 0.34697915153247866, 0.3697580702645849, -0.9949973005490104]])
V = np.array([[0.0026140108173807915, 0.6944345633371292, -0.5652773041221544, -0.35343275859595025, -0.03433664562735461, 0.08091670140460634, -0.0892103404240648, -0.1980300231087587], [0.2576248520364635, 0.06539948454957029, -0.35434557927644844, -0.03640441158856663, -0.7413593971475833, 0.0030001701455498278, 0.3713639451526768, 0.016947075929799594], [-0.5377309758940755, -0.02096760544900235, 0.40365084423895436, 0.5095417434602116, -0.45423293309175394, -0.5702820721334585, 0.6190313285414931, 0.7858326418298565], [0.7170730175523563, 0.7001885499108222, 0.4925926570601597, -0.7743826610421906, -0.16559112080190702, 0.6571136713106263, -0.6611900442465742, -0.2983796128216165], [0.36093529561820403, -0.15093011216763902, -0.38641849081949886, 0.1202443758222842, -0.4641758957921707, -0.4862339638412094, 0.1837342512310362, 0.5039182198056593]])
W = np.array([[0.7951356712114984, -0.07784905999497176, 0.08450253790371903, 0.006843070854248517, 0.2048617974624018, -1.523924051439455, 0.8830139483275325, 0.5211882387254724], [0.5093941381116157, -0.7659769028241413, -0.3653038243879763, -0.8496149079844891, 0.052715213787387104, 0.18251310702150852, 0.268561851999145, 0.9142889507799132], [0.021385010903070902, -0.4182776710107811, 0.26977388961992294, -1.1442626505742266, -1.0048448949104412, 0.34663597211489194, 1.2092826345430325, 0.8086175923533013], [-0.9015995943490751, 1.249123426342828, -0.5049639898080718, 2.545125440023137, 0.16782025096354364, -1.5011481522860137, 0.409842324079843, 0.27493076503176855], [0.9934580335307789, -0.10023212966102599, -0.4889278808326145, -2.6183798202363553, -0.4522780676075401, 1.1697194808175109, 0.8428489593111734, 0.2161166285673376]])


def _cp_factors_for(C):
    """Return (U, V, W) float64 with C[m,n,p] ~= sum_r U[m,r]V[n,r]W[p,r].

    Uses the embedded factors when C matches their reconstruction (the fixed
    real-CG tensor for l1=1, l2=2, L=2); otherwise fits a rank-8 CP
    decomposition to the given C at runtime via ALS with restarts.
    """
    C = np.asarray(C, dtype=np.float64)
    recon = np.einsum('mr,nr,pr->mnp', U, V, W)
    if np.abs(recon - C).max() < 1e-5 * max(1.0, np.abs(C).max()):
        return U, V, W

    def khatri(X, Y):
        return (X[:, None, :] * Y[None, :, :]).reshape(-1, X.shape[1])

    C1 = C.reshape(3, 25)
    C2 = C.transpose(1, 0, 2).reshape(5, 15)
    C3 = C.transpose(2, 0, 1).reshape(5, 15)
    best = None
    for seed in range(64):
        rng = np.random.default_rng(seed)
        u = rng.standard_normal((3, R))
        v = rng.standard_normal((5, R))
        w = rng.standard_normal((5, R))
        for _ in range(3000):
            u = C1 @ np.linalg.pinv(khatri(v, w).T)
            v = C2 @ np.linalg.pinv(khatri(u, w).T)
            w = C3 @ np.linalg.pinv(khatri(u, v).T)
        err = np.abs(np.einsum('mr,nr,pr->mnp', u, v, w) - C).max()
        if best is None or err < best[0]:
            best = (err, u, v, w)
        if err < 1e-9 * max(1.0, np.abs(C).max()):
            break
    err, u, v, w = best
    if err > 1e-5 * max(1.0, np.abs(C).max()):
        raise RuntimeError(f"runtime CP fit of C failed: absmax err {err}")
    su = np.linalg.norm(u, axis=0)
    sv = np.linalg.norm(v, axis=0)
    return u / su, v / sv, w * (su * sv)


def _build_weights(u, v, w):
    """WA [9,120] f16, WB [25,120] f16, WO [120,75] f32; q = r*15 + a*5 + b."""
    WA = np.zeros((9, 15 * R), np.float32)
    WB = np.zeros((25, 15 * R), np.float32)
    WO = np.zeros((15 * R, 75), np.float32)
    for r in range(R):
        for a in range(3):
            for b in range(5):
                q = r * 15 + a * 5 + b
                for m in range(3):
                    WA[m * 3 + a, q] = u[m, r]
                for n in range(5):
                    WB[n * 5 + b, q] = v[n, r]
                for p in range(5):
                    WO[q, p * 15 + a * 5 + b] = w[p, r]
    return WA.astype(np.float16), WB.astype(np.float16), WO


def _build_nc(WA, WB, WO, reps=1):
    import concourse.bass as bass
    import concourse.bacc as bacc
    import concourse.mybir as mybir
    from concourse import tile

    f16 = mybir.dt.float16
    f32 = mybir.dt.float32
    f32r = mybir.dt.float32r

    nc = bacc.Bacc()
    a_in = nc.declare_dram_parameter("a_pack", [9, NPAD], f16, isOutput=False)
    b_in = nc.declare_dram_parameter("b_pack", [25, NPAD], f16, isOutput=False)
    out_d = nc.declare_dram_parameter("out_t", [75, NPAD], f16, isOutput=True)
    wa_d = nc.inline_tensor(np.asarray(WA, np.float16), name="wa")
    wb_d = nc.inline_tensor(np.asarray(WB, np.float16), name="wb")
    wo_d = nc.inline_tensor(np.asarray(WO, np.float32), name="wo")

    with tile.TileContext(nc) as tc:
        with (
            tc.tile_pool(name="const", bufs=1) as cpool,
            tc.tile_pool(name="a", bufs=2) as a_pool,
            tc.tile_pool(name="b", bufs=2) as b_pool,
            tc.tile_pool(name="aus", bufs=2) as au_sb,
            tc.tile_pool(name="p", bufs=2) as p_pool,
            tc.tile_pool(name="ost", bufs=2) as ost_pool,
            tc.tile_pool(name="au_ps", bufs=2, space=bass.MemorySpace.PSUM) as au_ps,
            tc.tile_pool(name="bv_ps", bufs=2, space=bass.MemorySpace.PSUM) as bv_ps,
            tc.tile_pool(name="o_ps", bufs=2, space=bass.MemorySpace.PSUM) as o_ps,
        ):
            wa_t = cpool.tile([9, 15 * R], f16, tag="wa")
            wb_t = cpool.tile([25, 15 * R], f16, tag="wb")
            wo_t = cpool.tile([15 * R, 75], f32r, tag="wo")
            nc.sync.dma_start(wa_t[:], wa_d[:, :])
            nc.sync.dma_start(wb_t[:], wb_d[:, :])
            # SWDGE DMA casts f32 -> f32r (rounding) during the load
            nc.gpsimd.dma_start(wo_t[:], wo_d[:, :])

            import contextlib
            rep_ctx = (tc.For_i(0, reps, 1) if reps > 1
                       else contextlib.nullcontext())
            with rep_ctx:
              a_t = None
              b_t = None
              ost = None
              au_s = None
              bv_w = None
              for t in range(T):
                  k, j = divmod(t, CH)       # chunk index, tile-in-chunk
                  if j == 0:
                      a_t = a_pool.tile([9, CH * NT], f16, tag="a")
                      nc.sync.dma_start(a_t[:], a_in[:, CH * NT * k:CH * NT * (k + 1)])
                      b_t = b_pool.tile([25, CH * NT], f16, tag="b")
                      nc.sync.dma_start(b_t[:], b_in[:, CH * NT * k:CH * NT * (k + 1)])
                      ost = ost_pool.tile([75, CH * NT], f16, tag="ost")

                  # pairs are chunk-local: 21 tiles = 10 pairs + 1 single
                  half = j % 2
                  single = (j == CH - 1)
                  if half == 0:
                      bv_w = bv_ps.tile([15 * R, 2 * NT], f32, tag="bv")
                      au_s = au_sb.tile([15 * R, 2 * NT], f32, tag="aus")
                  col0 = NT * half
                  nc.tensor.matmul(
                      bv_w[:, col0:col0 + NT],
                      wb_t[:],
                      b_t[:, NT * j:NT * (j + 1)],
                      tile_position=(0, 0),
                  )
                  au = au_ps.tile([15 * R, NT], f32, tag="au")
                  nc.tensor.matmul(
                      au[:],
                      wa_t[:],
                      a_t[:, NT * j:NT * (j + 1)],
                      tile_position=(0, 0),
                  )
                  # copy balance: DVE takes the even-tile au copy (it also
                  # runs the pair mul); ACT takes the odd au copy + both
                  # out copies.
                  if half == 0 and not single:
                      nc.vector.tensor_copy(au_s[:, col0:col0 + NT], au[:])
                  else:
                      nc.scalar.copy(au_s[:, col0:col0 + NT], au[:])

                  if single:
                      p = p_pool.tile([15 * R, NT], f32r, tag="p")
                      nc.vector.tensor_mul(
                          p[:], au_s[:, 0:NT], bv_w[:, 0:NT])
                      o = o_ps.tile([75, NT], f32, tag="o")
                      nc.tensor.matmul(o[:], wo_t[:], p[:],
                                       tile_position=(0, 0))
                      nc.scalar.copy(ost[:, NT * j:NT * (j + 1)], o[:])
                  elif half == 1:
                      p = p_pool.tile([15 * R, 2 * NT], f32r, tag="p")
                      nc.vector.tensor_mul(p[:], au_s[:], bv_w[:])
                      o0 = o_ps.tile([75, NT], f32, tag="o")
                      nc.tensor.matmul(o0[:], wo_t[:], p[:, 0:NT],
                                       tile_position=(0, 0))
                      o1 = o_ps.tile([75, NT], f32, tag="o")
                      nc.tensor.matmul(o1[:], wo_t[:], p[:, NT:2 * NT],
                                       tile_position=(0, 0))
                      nc.scalar.copy(ost[:, NT * (j - 1):NT * j], o0[:])
                      nc.scalar.copy(ost[:, NT * j:NT * (j + 1)], o1[:])

                  if j == CH - 1:
                      nc.sync.dma_start(
                          out_d[:, CH * NT * k:CH * NT * (k + 1)], ost[:]
                      )
    nc.finalize()
    return nc


def _pack_inputs(A, B):
    """Per-core packed [9, NPAD] / [25, NPAD] fp16 arrays."""
    A9 = np.asarray(A, np.float32).reshape(N_ATOMS, 9)
    B25 = np.asarray(B, np.float32).reshape(N_ATOMS, 25)
    a_maps = []
    b_maps = []
    for c in range(NCORES):
        Ac = np.zeros((NPAD, 9), np.float16)
        Ac[:NPC] = A9[c * NPC:(c + 1) * NPC]
        a_maps.append(np.ascontiguousarray(Ac.T))
        Bc = np.zeros((NPAD, 25), np.float16)
        Bc[:NPC] = B25[c * NPC:(c + 1) * NPC]
        b_maps.append(np.ascontiguousarray(Bc.T))
    return a_maps, b_maps


_NC_CACHE = {}


def kernel(A, B, C):
    from concourse.bass_utils import run_bass_kernel_spmd

    A = np.asarray(A, dtype=np.float32)
    B = np.asarray(B, dtype=np.float32)
    C = np.asarray(C, dtype=np.float32)

    key = C.tobytes()
    if key not in _NC_CACHE:
        u, v, w = _cp_factors_for(C)
        WA, WB, WO = _build_weights(u, v, w)
        _NC_CACHE[key] = _build_nc(WA, WB, WO)
    nc = _NC_CACHE[key]

    a_maps, b_maps = _pack_inputs(A, B)
    in_maps = [{"a_pack": a_maps[c], "b_pack": b_maps[c]} for c in range(NCORES)]
    res = run_bass_kernel_spmd(nc, in_maps, list(range(NCORES)))
    outs = [res.results[c]["out_t"][:, :NPC] for c in range(NCORES)]
    full = np.concatenate(outs, axis=1)          # [75, 600000] fp16
    return np.ascontiguousarray(full.T).astype(np.float32).reshape(N_ATOMS, 5, 15)


if __name__ == "__main__":
    rng = np.random.default_rng(0)
    A = rng.standard_normal((N_ATOMS, 3, 3)).astype(np.float32)
    B = rng.standard_normal((N_ATOMS, 5, 5)).astype(np.float32)
    C = np.einsum('mr,nr,pr->mnp', U, V, W).astype(np.float32)
    out = kernel(A, B, C)
    print(out.shape, out.dtype)


# revision 6
# speedup vs baseline: 1.2621x; 1.2621x over previous
# Trainium2 Bass kernel for batched CG combine:
#   out[i, p, a, b] = sum_{m,n} A[i, m, a] * B[i, n, b] * C[m, n, p]
# A: (600000, 3, 3) f32, B: (600000, 5, 5) f32, C: (3, 5, 5) f32
# out: (600000, 5, 15) f32
#
# Algorithm: exact rank-8 CP decomposition C[m,n,p] = sum_r U[m,r] V[n,r] W[p,r].
# Per atom-tile (atoms on the free dim, NT=512 = one PSUM bank):
#   AU_E[(r,a,b), i] = sum_m (U[m,r] * dirac_a) A[(m,a), i]      (PE matmul, K=9)
#   BV_E[(r,a,b), i] = sum_n (V[n,r] * dirac_b) B[(n,b), i]      (PE matmul, K=25)
#   P = AU_E * BV_E                                              (DVE multiply)
#   out[(p,a,b), i] = sum_(r,a,b) (W[p,r] dirac_a dirac_b) P     (PE matmul, K=120)
#
# Key structure (vs. the 283us baseline):
#  - fp16 inputs and outputs over DMA (in 5.1MB + out 11.3MB per core vs
#    39.6MB), tightly packed.
#  - Even tiles load A at partitions 0-8 / B at 32-56; odd tiles at 64-72 /
#    96-120 (4 co-resident PE stationaries). The SDMA partition swizzle maps
#    p and p+32 to the same engine, so bases {0,64} and {32,96} engage
#    disjoint engine sets - twice the DMA parallelism of a tight 9/25-row
#    layout.
#  - Input DMAs issue from the (idle) GPSIMD SWDGE ring; output DMAs from
#    the SP HWDGE ring, so output-DMA dependency waits cannot stall input
#    prefetch (both on one sequencer was a measured bottleneck).
#  - NT=512 fills a PSUM bank exactly; atoms padded to 147 tiles/core.
#  - BV lands in a 2-bank PSUM tile [120, 1024]; ONE DVE tensor_mul per
#    PAIR of tiles (FD=1024) halves the per-op fixed cost of the multiply.
#  - Copy assignment balanced per the engine cost model: DVE (0.96GHz,
#    captive to the mul) takes one au-copy per pair; ACT (1.2GHz) takes
#    the other au-copy and both out-copies.
#  - PSUM budget: au 2 banks + bv-wide 4 banks + o 2 banks = 8 exactly.
#
# Sharding: data-parallel over atoms, 75000 per core across 8 cores.

import numpy as np

N_ATOMS = 600000
NCORES = 8
NPC = N_ATOMS // NCORES   # 75000 real atoms per core
NT = 512                  # atoms per tile = one PSUM bank of f32
T = 147                   # tiles per core
NPAD = NT * T             # 75264 padded atoms per core
CH = 21                   # tiles per chunk (147 = 7 * 21)
NCHUNK = T // CH          # 7 chunks per core
NE = (CH + 1) // 2        # even tiles per chunk (11)
NO = CH // 2              # odd tiles per chunk (10)

R = 8  # CP rank (exact for this CG tensor)

U = np.array([[0.2419016152442985, 0.6625062831986197, -0.8309374270990885, 0.3998142823675103, -0.5651140448972596, -0.34640840162110975, 0.7646485241540064, -0.0981640650113134], [0.9679329076741274, -0.6672684032643771, -0.5353370910241713, -0.9127024843358726, 0.26799289625560263, 0.8715541794335616, -0.5278177753574712, -0.018552310924435454], [0.06774581008230969, 0.3403502647675755, 0.1515163067782647, -0.08439617705843598, 0.7802729803193187, 0.34697915153247866, 0.3697580702645849, -0.9949973005490104]])
V = np.array([[0.0026140108173807915, 0.6944345633371292, -0.5652773041221544, -0.35343275859595025, -0.03433664562735461, 0.08091670140460634, -0.0892103404240648, -0.1980300231087587], [0.2576248520364635, 0.06539948454957029, -0.35434557927644844, -0.03640441158856663, -0.7413593971475833, 0.0030001701455498278, 0.3713639451526768, 0.016947075929799594], [-0.5377309758940755, -0.02096760544900235, 0.40365084423895436, 0.5095417434602116, -0.45423293309175394, -0.5702820721334585, 0.6190313285414931, 0.7858326418298565], [0.7170730175523563, 0.7001885499108222, 0.4925926570601597, -0.7743826610421906, -0.16559112080190702, 0.6571136713106263, -0.6611900442465742, -0.2983796128216165], [0.36093529561820403, -0.15093011216763902, -0.38641849081949886, 0.1202443758222842, -0.4641758957921707, -0.4862339638412094, 0.1837342512310362, 0.5039182198056593]])
W = np.array([[0.7951356712114984, -0.07784905999497176, 0.08450253790371903, 0.006843070854248517, 0.2048617974624018, -1.523924051439455, 0.8830139483275325, 0.5211882387254724], [0.5093941381116157, -0.7659769028241413, -0.3653038243879763, -0.8496149079844891, 0.052715213787387104, 0.18251310702150852, 0.268561851999145, 0.9142889507799132], [0.021385010903070902, -0.4182776710107811, 0.26977388961992294, -1.1442626505742266, -1.0048448949104412, 0.34663597211489194, 1.2092826345430325, 0.8086175923533013], [-0.9015995943490751, 1.249123426342828, -0.5049639898080718, 2.545125440023137, 0.16782025096354364, -1.5011481522860137, 0.409842324079843, 0.27493076503176855], [0.9934580335307789, -0.10023212966102599, -0.4889278808326145, -2.6183798202363553, -0.4522780676075401, 1.1697194808175109, 0.8428489593111734, 0.2161166285673376]])


def _cp_factors_for(C):
    """Return (U, V, W) float64 with C[m,n,p] ~= sum_r U[m,r]V[n,r]W[p,r].

    Uses the embedded factors when C matches their reconstruction (the fixed
    real-CG tensor for l1=1, l2=2, L=2); otherwise fits a rank-8 CP
    decomposition to the given C at runtime via ALS with restarts.
    """
    C = np.asarray(C, dtype=np.float64)
    recon = np.einsum('mr,nr,pr->mnp', U, V, W)
    if np.abs(recon - C).max() < 1e-5 * max(1.0, np.abs(C).max()):
        return U, V, W

    def khatri(X, Y):
        return (X[:, None, :] * Y[None, :, :]).reshape(-1, X.shape[1])

    C1 = C.reshape(3, 25)
    C2 = C.transpose(1, 0, 2).reshape(5, 15)
    C3 = C.transpose(2, 0, 1).reshape(5, 15)
    best = None
    for seed in range(64):
        rng = np.random.default_rng(seed)
        u = rng.standard_normal((3, R))
        v = rng.standard_normal((5, R))
        w = rng.standard_normal((5, R))
        for _ in range(3000):
            u = C1 @ np.linalg.pinv(khatri(v, w).T)
            v = C2 @ np.linalg.pinv(khatri(u, w).T)
            w = C3 @ np.linalg.pinv(khatri(u, v).T)
        err = np.abs(np.einsum('mr,nr,pr->mnp', u, v, w) - C).max()
        if best is None or err < best[0]:
            best = (err, u, v, w)
        if err < 1e-9 * max(1.0, np.abs(C).max()):
            break
    err, u, v, w = best
    if err > 1e-5 * max(1.0, np.abs(C).max()):
        raise RuntimeError(f"runtime CP fit of C failed: absmax err {err}")
    su = np.linalg.norm(u, axis=0)
    sv = np.linalg.norm(v, axis=0)
    return u / su, v / sv, w * (su * sv)


def _build_weights(u, v, w):
    """WA [9,120] f16, WB [25,120] f16, WO [120,75] f32; q = r*15 + a*5 + b."""
    WA = np.zeros((9, 15 * R), np.float32)
    WB = np.zeros((25, 15 * R), np.float32)
    WO = np.zeros((15 * R, 75), np.float32)
    for r in range(R):
        for a in range(3):
            for b in range(5):
                q = r * 15 + a * 5 + b
                for m in range(3):
                    WA[m * 3 + a, q] = u[m, r]
                for n in range(5):
                    WB[n * 5 + b, q] = v[n, r]
                for p in range(5):
                    WO[q, p * 15 + a * 5 + b] = w[p, r]
    return WA.astype(np.float16), WB.astype(np.float16), WO


def _build_nc(WA, WB, WO, reps=1, wide_o=False):
    import concourse.bass as bass
    import concourse.bacc as bacc
    import concourse.mybir as mybir
    from concourse import tile

    f16 = mybir.dt.float16
    f32 = mybir.dt.float32
    f32r = mybir.dt.float32r

    nc = bacc.Bacc()
    ae_in = nc.declare_dram_parameter("ae", [9, NCHUNK * NE * NT], f16, isOutput=False)
    ao_in = nc.declare_dram_parameter("ao", [9, NCHUNK * NO * NT], f16, isOutput=False)
    be_in = nc.declare_dram_parameter("be", [25, NCHUNK * NE * NT], f16, isOutput=False)
    bo_in = nc.declare_dram_parameter("bo", [25, NCHUNK * NO * NT], f16, isOutput=False)
    out_d = nc.declare_dram_parameter("out_t", [75, NPAD], f16, isOutput=True)
    # stationary sources must sit at the PE-row partitions of their
    # tile_position: WA at 0-8 and 64-72, WB at 32-56 and 96-120.
    WAB = np.zeros((121, 15 * R), np.float16)
    WAB[0:9] = WA
    WAB[32:57] = WB
    WAB[64:73] = WA
    WAB[96:121] = WB
    wab_d = nc.inline_tensor(WAB, name="wab")
    wo_d = nc.inline_tensor(np.asarray(WO, np.float32), name="wo")

    with tile.TileContext(nc) as tc:
        with (
            tc.tile_pool(name="const", bufs=1) as cpool,
            tc.tile_pool(name="ab", bufs=2) as ab_pool,
            tc.tile_pool(name="aus", bufs=3) as au_sb,
            tc.tile_pool(name="p", bufs=3) as p_pool,
            tc.tile_pool(name="ost", bufs=2) as ost_pool,
            tc.tile_pool(name="au_ps", bufs=2, space=bass.MemorySpace.PSUM) as au_ps,
            tc.tile_pool(name="bv_ps", bufs=2, space=bass.MemorySpace.PSUM) as bv_ps,
            tc.tile_pool(name="o_ps", bufs=(1 if wide_o else 2), space=bass.MemorySpace.PSUM) as o_ps,
        ):
            wab_t = cpool.tile([121, 15 * R], f16, tag="wab")
            wo_t = cpool.tile([15 * R, 75], f32r, tag="wo")
            nc.sync.dma_start(wab_t[:], wab_d[:, :])
            # SWDGE DMA casts f32 -> f32r (rounding) during the load
            nc.gpsimd.dma_start(wo_t[:], wo_d[:, :])

            import contextlib
            rep_ctx = (tc.For_i(0, reps, 1) if reps > 1
                       else contextlib.nullcontext())
            with rep_ctx:
              ab_t = None
              ost = None
              au_s = None
              bv_w = None
              for t in range(T):
                  k, j = divmod(t, CH)       # chunk index, tile-in-chunk
                  if j == 0:
                      # one SBUF tile, 4 partition-base regions, 4 input DMAs
                      # on the SWDGE (gpsimd) ring
                      ab_t = ab_pool.tile([121, NE * NT], f16, tag="ab")
                      nc.gpsimd.dma_start(
                          ab_t[0:9, 0:NE * NT],
                          ae_in[:, NE * NT * k:NE * NT * (k + 1)])
                      nc.gpsimd.dma_start(
                          ab_t[32:57, 0:NE * NT],
                          be_in[:, NE * NT * k:NE * NT * (k + 1)])
                      nc.gpsimd.dma_start(
                          ab_t[64:73, 0:NO * NT],
                          ao_in[:, NO * NT * k:NO * NT * (k + 1)])
                      nc.gpsimd.dma_start(
                          ab_t[96:121, 0:NO * NT],
                          bo_in[:, NO * NT * k:NO * NT * (k + 1)])
                      ost = ost_pool.tile([75, CH * NT], f16, tag="ost")

                  # pairs are chunk-local: 21 tiles = 10 pairs + 1 single
                  half = j % 2
                  single = (j == CH - 1)
                  g = j // 2                 # pair index / even-slot index
                  if half == 0:
                      arow, brow, tp = 0, 32, 0
                      icol = NT * g
                  else:
                      arow, brow, tp = 64, 96, 64
                      icol = NT * g
                  if half == 0:
                      bv_w = bv_ps.tile([15 * R, 2 * NT], f32, tag="bv")
                      au_s = au_sb.tile([15 * R, 2 * NT], f32, tag="aus")
                  col0 = NT * half
                  nc.tensor.matmul(
                      bv_w[:, col0:col0 + NT],
                      wab_t[brow:brow + 25, :],
                      ab_t[brow:brow + 25, icol:icol + NT],
                      tile_position=(brow, 0),
                  )
                  au = au_ps.tile([15 * R, NT], f32, tag="au")
                  nc.tensor.matmul(
                      au[:],
                      wab_t[arow:arow + 9, :],
                      ab_t[arow:arow + 9, icol:icol + NT],
                      tile_position=(arow, 0),
                  )
                  # copy balance: DVE takes the even-tile au copy (it also
                  # runs the pair mul); ACT takes the odd au copy + both
                  # out copies.
                  if half == 0 and not single:
                      nc.vector.tensor_copy(au_s[:, col0:col0 + NT], au[:])
                  else:
                      nc.scalar.copy(au_s[:, col0:col0 + NT], au[:])

                  if single:
                      p = p_pool.tile([15 * R, NT], f32r, tag="p")
                      nc.vector.tensor_mul(
                          p[:], au_s[:, 0:NT], bv_w[:, 0:NT])
                      o = o_ps.tile([75, NT], f32, tag="o")
                      nc.tensor.matmul(o[:], wo_t[:], p[:],
                                       tile_position=(0, 0))
                      nc.scalar.copy(ost[:, NT * j:NT * (j + 1)], o[:])
                  elif half == 1:
                      p = p_pool.tile([15 * R, 2 * NT], f32r, tag="p")
                      nc.vector.tensor_mul(p[:], au_s[:], bv_w[:])
                      if wide_o:
                          ow = o_ps.tile([75, 2 * NT], f32, tag="o")
                          nc.tensor.matmul(ow[:, 0:NT], wo_t[:], p[:, 0:NT],
                                           tile_position=(0, 0))
                          nc.tensor.matmul(ow[:, NT:2 * NT], wo_t[:],
                                           p[:, NT:2 * NT],
                                           tile_position=(0, 0))
                          nc.scalar.copy(
                              ost[:, NT * (j - 1):NT * (j + 1)], ow[:])
                      else:
                          o0 = o_ps.tile([75, NT], f32, tag="o")
                          nc.tensor.matmul(o0[:], wo_t[:], p[:, 0:NT],
                                           tile_position=(0, 0))
                          o1 = o_ps.tile([75, NT], f32, tag="o")
                          nc.tensor.matmul(o1[:], wo_t[:], p[:, NT:2 * NT],
                                           tile_position=(0, 0))
                          nc.scalar.copy(ost[:, NT * (j - 1):NT * j], o0[:])
                          nc.scalar.copy(ost[:, NT * j:NT * (j + 1)], o1[:])

                  if j == CH - 1:
                      nc.sync.dma_start(
                          out_d[:, CH * NT * k:CH * NT * (k + 1)], ost[:]
                      )
    nc.finalize()
    return nc


def _pack_inputs(A, B):
    """Per-core even/odd packed fp16 arrays: ae [9, 7*11*512],
    ao [9, 7*10*512], be [25, ...], bo [25, ...]."""
    A9 = np.asarray(A, np.float32).reshape(N_ATOMS, 9)
    B25 = np.asarray(B, np.float32).reshape(N_ATOMS, 25)
    maps = []
    for c in range(NCORES):
        Ac = np.zeros((NPAD, 9), np.float16)
        Ac[:NPC] = A9[c * NPC:(c + 1) * NPC]
        Bc = np.zeros((NPAD, 25), np.float16)
        Bc[:NPC] = B25[c * NPC:(c + 1) * NPC]
        At = Ac.reshape(NCHUNK, CH, NT, 9)
        Bt = Bc.reshape(NCHUNK, CH, NT, 25)
        ae = At[:, 0::2].reshape(NCHUNK * NE * NT, 9).T
        ao = At[:, 1::2].reshape(NCHUNK * NO * NT, 9).T
        be = Bt[:, 0::2].reshape(NCHUNK * NE * NT, 25).T
        bo = Bt[:, 1::2].reshape(NCHUNK * NO * NT, 25).T
        maps.append({
            "ae": np.ascontiguousarray(ae),
            "ao": np.ascontiguousarray(ao),
            "be": np.ascontiguousarray(be),
            "bo": np.ascontiguousarray(bo),
        })
    return maps


_NC_CACHE = {}


def kernel(A, B, C):
    from concourse.bass_utils import run_bass_kernel_spmd

    A = np.asarray(A, dtype=np.float32)
    B = np.asarray(B, dtype=np.float32)
    C = np.asarray(C, dtype=np.float32)

    key = C.tobytes()
    if key not in _NC_CACHE:
        u, v, w = _cp_factors_for(C)
        WA, WB, WO = _build_weights(u, v, w)
        _NC_CACHE[key] = _build_nc(WA, WB, WO, wide_o=True)
    nc = _NC_CACHE[key]

    in_maps = _pack_inputs(A, B)
    res = run_bass_kernel_spmd(nc, in_maps, list(range(NCORES)))
    outs = [res.results[c]["out_t"][:, :NPC] for c in range(NCORES)]
    full = np.concatenate(outs, axis=1)          # [75, 600000] fp16
    return np.ascontiguousarray(full.T).astype(np.float32).reshape(N_ATOMS, 5, 15)


if __name__ == "__main__":
    rng = np.random.default_rng(0)
    A = rng.standard_normal((N_ATOMS, 3, 3)).astype(np.float32)
    B = rng.standard_normal((N_ATOMS, 5, 5)).astype(np.float32)
    C = np.einsum('mr,nr,pr->mnp', U, V, W).astype(np.float32)
    out = kernel(A, B, C)
    print(out.shape, out.dtype)
